# revision 8
# baseline (speedup 1.0000x reference)
"""L21 norm kernel for Trainium2 (Bass/Tile), 8-core SPMD.

Computes sum_j sqrt(sum_i S[i,j]^2) for S of shape [8192, 16384] fp32.

Sharding: S is split along columns into 8 shards of [8192, 2048] (one per
NeuronCore). Each core computes the sum of its columns' L2 norms as a
scalar; the host sums the 8 partial scalars.

Per-core dataflow (memory-bound; 64 MiB HBM read per core, ~187 us floor
at 358 GB/s per-NC):
  - 32 tiles of [128 partitions, 2 rows, 2048 cols] fp32 (2 MiB HWDGE
    DMAs; each partition's slice is 16 KiB contiguous in DRAM).
  - ACT engine: square with bf16 output (also the dtype cast for PE).
  - Partition-axis reduction is split so neither engine paces the DMA
    stream: per tile, row-slice q=0 goes to PE (ones[128,1]^T @ sq
    matmuls accumulating into PSUM [1,2048] fp32) and row-slice q=1 is
    accumulated on DVE into a bf16 [128,2048] accumulator (2x mode).
    The DVE accumulator is folded into PSUM via 4 matmuls near the end;
    the last tile sends both row-slices to PE to keep the tail short.
  - Epilogue: ACT sqrt (PSUM -> SBUF), DVE free-axis reduce_sum -> [1,1],
    DMA to DRAM.
"""

import numpy as np

# Full problem shape (hardcoded per the harness contract).
R = 8192          # rows
C_FULL = 16384    # columns
N_CORES = 8
C = C_FULL // N_CORES  # 2048 columns per core
P = 128           # SBUF partitions
Q = 2             # rows per partition per tile (16 KiB contiguous DRAM)
T = R // (P * Q)  # tiles per core (32)
NBLK = 512        # matmul moving free dim (one PSUM bank of fp32)
# DVE-accumulator fold points (after the add at tile t) and the tiles that
# restart the accumulator with a copy. The final fold at T-3 covers the
# remaining adds; the last full tile (T-2) goes straight to PE.
FOLD_TILES = (7, 15, 23, T - 3)
RESET_TILES = (8, 16, 24)

_cached = None


def _build():
    """Build + schedule the per-core Bass program. Returns the Bacc object."""
    import concourse.bacc as bacc
    import concourse.tile as tile
    from concourse import mybir

    nc = bacc.Bacc(
        "TRN2",
        target_bir_lowering=False,
        debug=False,
        enable_asserts=False,
        num_devices=N_CORES,
    )

    s_dram = nc.dram_tensor("S", [R, C], mybir.dt.float32, kind="ExternalInput")
    out_dram = nc.dram_tensor("out", [1, 1], mybir.dt.float32, kind="ExternalOutput")

    s_ap = s_dram.ap()
    out_ap = out_dram.ap()

    # [T, P, Q, C]: tile t covers rows [t*P*Q, (t+1)*P*Q); partition p holds
    # Q consecutive rows -> 16 KiB contiguous DRAM per (t, p) descriptor.
    # The last full tile's worth of rows is instead handled as two
    # [P, 1, C] sub-tiles to shorten the serial epilogue chain.
    TF = T - 1  # number of full tiles (0..TF-1)
    s_view = s_ap.rearrange("(t p q) c -> t p q c", p=P, q=Q)
    s_tail = s_ap[TF * P * Q :, :].rearrange("(s p) c -> s p c", p=P)  # [Q, P, C]

    with tile.TileContext(nc) as tc:
        with (
            tc.tile_pool(name="io", bufs=4) as io_pool,
            tc.tile_pool(name="sqp", bufs=3) as sq_pool,
            tc.tile_pool(name="const", bufs=1) as const_pool,
            tc.tile_pool(name="ps", bufs=1, space="PSUM") as ps_pool,
            tc.tile_pool(name="fin", bufs=1) as fin_pool,
        ):
            # Input DMAs alternate between the two HWDGE rings (SP via
            # nc.sync, ACT via nc.scalar) to keep more requests in flight
            # at the HBM arbiter.
            def in_dma(t, out, in_):
                eng = nc.sync if t % 2 == 0 else nc.scalar
                eng.dma_start(out=out, in_=in_)

            # First input DMA before any const setup so streaming starts as
            # early as possible.
            x0 = io_pool.tile([P, Q, C], mybir.dt.float32, tag="x")
            in_dma(0, x0, s_view[0])

            ones = const_pool.tile([P, 1], mybir.dt.bfloat16)
            nc.vector.memset(ones, 1.0)

            # DVE-side accumulator for q=1 row-slices.
            acc = const_pool.tile([P, C], mybir.dt.bfloat16)

            # Per-column sum of squares (4 PSUM banks).
            colsq = ps_pool.tile([1, C], mybir.dt.float32)

            # Dummy sqrt: pulls the sqrt ACT-table load out of the tail.
            warm = const_pool.tile([1, 1], mybir.dt.float32)
            nc.scalar.sqrt(out=warm, in_=ones[0:1, :])

            def pe_reduce(src, first, last):
                for b in range(C // NBLK):
                    nc.tensor.matmul(
                        colsq[:, b * NBLK : (b + 1) * NBLK],
                        ones,
                        src[:, b * NBLK : (b + 1) * NBLK],
                        start=first,
                        stop=(last and b == C // NBLK - 1),
                    )

            for t in range(TF):
                if t == 0:
                    x_tile = x0
                else:
                    x_tile = io_pool.tile([P, Q, C], mybir.dt.float32, tag="x")
                    in_dma(t, x_tile, s_view[t])

                sq = sq_pool.tile([P, Q, C], mybir.dt.bfloat16, tag="sq")
                nc.scalar.square(out=sq, in_=x_tile)

                # q=0 row-slice -> PE psum accumulate.
                pe_reduce(sq[:, 0, :], first=(t == 0), last=False)

                # q=1 row-slice -> DVE bf16 accumulator; the last full tile
                # goes to PE (the accumulator is already folded by then).
                # The accumulator is folded into PSUM every 8 tiles to keep
                # the bf16 accumulation chains short (less rounding error);
                # PE has plenty of slack mid-stream.
                if t == 0 or t in RESET_TILES:
                    nc.vector.tensor_copy(acc, sq[:, 1, :])
                elif t < TF - 1:
                    nc.vector.tensor_add(acc, acc, sq[:, 1, :])
                else:
                    pe_reduce(sq[:, 1, :], first=False, last=False)

                if t in FOLD_TILES:
                    pe_reduce(acc, first=False, last=False)

            # Tail: two small [P, 1, C] sub-tiles keep the post-last-byte
            # chain short (small square, 4 matmuls).
            for s in range(Q):
                xs_tile = io_pool.tile([P, 1, C], mybir.dt.float32, tag="xs")
                in_dma(s, xs_tile[:, 0, :], s_tail[s])
                sqs = sq_pool.tile([P, 1, C], mybir.dt.bfloat16, tag="sqs")
                nc.scalar.square(out=sqs, in_=xs_tile)
                pe_reduce(sqs[:, 0, :], first=False, last=(s == Q - 1))

            # Per-block sqrt + partial reduce pipeline behind the last MMs.
            norms = fin_pool.tile([1, C], mybir.dt.float32)
            part = fin_pool.tile([1, C // NBLK], mybir.dt.float32)
            for b in range(C // NBLK):
                blk = slice(b * NBLK, (b + 1) * NBLK)
                nc.scalar.sqrt(out=norms[:, blk], in_=colsq[:, blk])
                nc.vector.reduce_sum(
                    out=part[:, b : b + 1], in_=norms[:, blk], axis=mybir.AxisListType.X
                )

            total = fin_pool.tile([1, 1], mybir.dt.float32)
            nc.vector.reduce_sum(out=total, in_=part, axis=mybir.AxisListType.X)

            nc.sync.dma_start(out=out_ap, in_=total)

    nc.compile()
    return nc


def _get_nc():
    global _cached
    if _cached is None:
        _cached = _build()
    return _cached


def _run(S: np.ndarray, trace: bool = False):
    from concourse import bass_utils

    assert S.shape == (R, C_FULL), S.shape
    S = np.ascontiguousarray(np.asarray(S, dtype=np.float32))

    nc = _get_nc()
    in_maps = [
        {"S": np.ascontiguousarray(S[:, i * C : (i + 1) * C])} for i in range(N_CORES)
    ]
    try:
        res = bass_utils.run_bass_kernel_spmd(
            nc, in_maps, core_ids=list(range(N_CORES)), trace=trace
        )
    except Exception:
        # One retry: transient NRT/device hiccups (e.g. a wedged core from a
        # previous process) are recoverable on re-execution.
        res = bass_utils.run_bass_kernel_spmd(
            nc, in_maps, core_ids=list(range(N_CORES)), trace=trace
        )
    partials = np.array(
        [res.results[i]["out"][0, 0] for i in range(N_CORES)], dtype=np.float64
    )
    out = np.float32(partials.sum())
    return out, res


def kernel(S: np.ndarray) -> np.ndarray:
    out, _ = _run(S, trace=False)
    return np.asarray(out, dtype=np.float32)


def run_traced(S: np.ndarray):
    """For test.py: returns (output, BassKernelResults) with NTFF trace."""
    return _run(S, trace=True)


# revision 9
# speedup vs baseline: 1.0385x; 1.0385x over previous
"""L21 norm kernel for Trainium2 (Bass/Tile), 8-core SPMD.

Computes sum_j sqrt(sum_i S[i,j]^2) for S of shape [8192, 16384] fp32.

Sharding: S is split along columns into 8 shards of [8192, 2048] (one per
NeuronCore). Each core computes the sum of its columns' L2 norms as a
scalar; the host sums the 8 partial scalars.

Per-core dataflow (memory-bound; 64 MiB HBM read per core, ~187 us floor
at 358 GB/s per-NC):
  - 32 tiles of [128 partitions, 2 rows, 2048 cols] fp32 (2 MiB HWDGE
    DMAs; each partition's slice is 16 KiB contiguous in DRAM).
  - ACT engine: square with bf16 output (also the dtype cast for PE).
  - Partition-axis reduction is split so neither engine paces the DMA
    stream: per tile, row-slice q=0 goes to PE (ones[128,1]^T @ sq
    matmuls accumulating into PSUM [1,2048] fp32) and row-slice q=1 is
    accumulated on DVE into a bf16 [128,2048] accumulator (2x mode).
    The DVE accumulator is folded into PSUM via 4 matmuls near the end;
    the last tile sends both row-slices to PE to keep the tail short.
  - Epilogue: ACT sqrt (PSUM -> SBUF), DVE free-axis reduce_sum -> [1,1],
    DMA to DRAM.
"""

import numpy as np

# Full problem shape (hardcoded per the harness contract).
R = 8192          # rows
C_FULL = 16384    # columns
N_CORES = 8
C = C_FULL // N_CORES  # 2048 columns per core
P = 128           # SBUF partitions
Q = 2             # rows per partition per tile (16 KiB contiguous DRAM)
T = R // (P * Q)  # tiles per core (32)
NBLK = 512        # matmul moving free dim (one PSUM bank of fp32)
# DVE-accumulator fold points (after the add at tile t) and the tiles that
# restart the accumulator with a copy. The final fold at T-3 covers the
# remaining adds; the last full tile (T-2) goes straight to PE.
FOLD_TILES = (7, 15, 23, T - 3)
RESET_TILES = (8, 16, 24)

_cached = None


def _build():
    """Build + schedule the per-core Bass program. Returns the Bacc object."""
    import concourse.bacc as bacc
    import concourse.tile as tile
    from concourse import mybir

    nc = bacc.Bacc(
        "TRN2",
        target_bir_lowering=False,
        debug=False,
        enable_asserts=False,
        num_devices=N_CORES,
    )

    s_dram = nc.dram_tensor("S", [R, C], mybir.dt.float32, kind="ExternalInput")
    out_dram = nc.dram_tensor("out", [1, 1], mybir.dt.float32, kind="ExternalOutput")

    s_ap = s_dram.ap()
    out_ap = out_dram.ap()

    # [T, P, Q, C]: tile t covers rows [t*P*Q, (t+1)*P*Q); partition p holds
    # Q consecutive rows -> 16 KiB contiguous DRAM per (t, p) descriptor.
    # The last full tile's worth of rows is instead handled as two
    # [P, 1, C] sub-tiles to shorten the serial epilogue chain.
    TF = T - 1  # number of full tiles (0..TF-1)
    s_view = s_ap.rearrange("(t p q) c -> t p q c", p=P, q=Q)
    s_tail = s_ap[TF * P * Q :, :].rearrange("(s p) c -> s p c", p=P)  # [Q, P, C]

    with tile.TileContext(nc) as tc:
        with (
            tc.tile_pool(name="io", bufs=6) as io_pool,
            tc.tile_pool(name="sqp", bufs=3) as sq_pool,
            tc.tile_pool(name="const", bufs=1) as const_pool,
            tc.tile_pool(name="ps", bufs=1, space="PSUM") as ps_pool,
            tc.tile_pool(name="fin", bufs=1) as fin_pool,
        ):
            # First input DMA before any const setup so streaming starts as
            # early as possible.
            x0 = io_pool.tile([P, Q, C], mybir.dt.float32, tag="x")
            nc.sync.dma_start(out=x0, in_=s_view[0])

            ones = const_pool.tile([P, 1], mybir.dt.bfloat16)
            nc.vector.memset(ones, 1.0)

            # DVE-side accumulator for q=1 row-slices.
            acc = const_pool.tile([P, C], mybir.dt.bfloat16)

            # Per-column sum of squares (4 PSUM banks).
            colsq = ps_pool.tile([1, C], mybir.dt.float32)

            # Dummy sqrt: pulls the sqrt ACT-table load out of the tail.
            warm = const_pool.tile([1, 1], mybir.dt.float32)
            nc.scalar.sqrt(out=warm, in_=ones[0:1, :])

            def pe_reduce(src, first, last):
                for b in range(C // NBLK):
                    nc.tensor.matmul(
                        colsq[:, b * NBLK : (b + 1) * NBLK],
                        ones,
                        src[:, b * NBLK : (b + 1) * NBLK],
                        start=first,
                        stop=(last and b == C // NBLK - 1),
                    )

            for t in range(TF):
                if t == 0:
                    x_tile = x0
                else:
                    x_tile = io_pool.tile([P, Q, C], mybir.dt.float32, tag="x")
                    nc.sync.dma_start(out=x_tile, in_=s_view[t])

                sq = sq_pool.tile([P, Q, C], mybir.dt.bfloat16, tag="sq")
                nc.scalar.square(out=sq, in_=x_tile)

                # q=0 row-slice -> PE psum accumulate.
                pe_reduce(sq[:, 0, :], first=(t == 0), last=False)

                # q=1 row-slice -> DVE bf16 accumulator; the last full tile
                # goes to PE (the accumulator is already folded by then).
                # The accumulator is folded into PSUM every 8 tiles to keep
                # the bf16 accumulation chains short (less rounding error);
                # PE has plenty of slack mid-stream.
                if t == 0 or t in RESET_TILES:
                    nc.vector.tensor_copy(acc, sq[:, 1, :])
                elif t < TF - 1:
                    nc.vector.tensor_add(acc, acc, sq[:, 1, :])
                else:
                    pe_reduce(sq[:, 1, :], first=False, last=False)

                if t in FOLD_TILES:
                    pe_reduce(acc, first=False, last=False)

            # Tail: two small [P, 1, C] sub-tiles keep the post-last-byte
            # chain short (small square, 4 matmuls).
            for s in range(Q):
                xs_tile = io_pool.tile([P, 1, C], mybir.dt.float32, tag="xs")
                nc.sync.dma_start(out=xs_tile[:, 0, :], in_=s_tail[s])
                sqs = sq_pool.tile([P, 1, C], mybir.dt.bfloat16, tag="sqs")
                nc.scalar.square(out=sqs, in_=xs_tile)
                pe_reduce(sqs[:, 0, :], first=False, last=(s == Q - 1))

            # Per-block sqrt + partial reduce pipeline behind the last MMs.
            norms = fin_pool.tile([1, C], mybir.dt.float32)
            part = fin_pool.tile([1, C // NBLK], mybir.dt.float32)
            for b in range(C // NBLK):
                blk = slice(b * NBLK, (b + 1) * NBLK)
                nc.scalar.sqrt(out=norms[:, blk], in_=colsq[:, blk])
                nc.vector.reduce_sum(
                    out=part[:, b : b + 1], in_=norms[:, blk], axis=mybir.AxisListType.X
                )

            total = fin_pool.tile([1, 1], mybir.dt.float32)
            nc.vector.reduce_sum(out=total, in_=part, axis=mybir.AxisListType.X)

            nc.sync.dma_start(out=out_ap, in_=total)

    nc.compile()
    return nc


def _get_nc():
    global _cached
    if _cached is None:
        _cached = _build()
    return _cached


def _run(S: np.ndarray, trace: bool = False):
    from concourse import bass_utils

    assert S.shape == (R, C_FULL), S.shape
    S = np.ascontiguousarray(np.asarray(S, dtype=np.float32))

    nc = _get_nc()
    in_maps = [
        {"S": np.ascontiguousarray(S[:, i * C : (i + 1) * C])} for i in range(N_CORES)
    ]
    try:
        res = bass_utils.run_bass_kernel_spmd(
            nc, in_maps, core_ids=list(range(N_CORES)), trace=trace
        )
    except Exception:
        # One retry: transient NRT/device hiccups (e.g. a wedged core from a
        # previous process) are recoverable on re-execution.
        res = bass_utils.run_bass_kernel_spmd(
            nc, in_maps, core_ids=list(range(N_CORES)), trace=trace
        )
    partials = np.array(
        [res.results[i]["out"][0, 0] for i in range(N_CORES)], dtype=np.float64
    )
    out = np.float32(partials.sum())
    return out, res


def kernel(S: np.ndarray) -> np.ndarray:
    out, _ = _run(S, trace=False)
    return np.asarray(out, dtype=np.float32)


def run_traced(S: np.ndarray):
    """For test.py: returns (output, BassKernelResults) with NTFF trace."""
    return _run(S, trace=True)


# revision 10
# speedup vs baseline: 1.2376x; 1.1917x over previous
"""L21 norm kernel for Trainium2 (Bass/Tile), 8-core SPMD.

Computes sum_j sqrt(sum_i S[i,j]^2) for S of shape [8192, 16384] fp32.

Sharding: S is split along columns into 8 shards of [8192, 2048] (one per
NeuronCore). Each core computes the sum of its columns' L2 norms as a
scalar; the host sums the 8 partial scalars.

Per-core dataflow (memory-bound; 64 MiB HBM read per core):
  - Bulk: 15 tiles of [128 partitions, 4 rows, 2048 cols] fp32 (4 MiB
    HWDGE DMAs; each partition's slice is 32 KiB contiguous in DRAM),
    then one [128, 2, 2048] tile and two [128, 1, 2048] sub-tiles so the
    post-last-byte chain stays short.
  - ACT engine: square with bf16 output (also the dtype cast for PE).
  - Partition-axis reduction is split so neither engine paces the DMA
    stream: per bulk tile, row-slices q=0,1 go to PE (ones[128,1]^T @ sq
    matmuls accumulating into PSUM [1,2048] fp32) and q=2,3 are
    accumulated on DVE into a bf16 [128,2048] accumulator (2x mode),
    folded into PSUM every 5 tiles (short bf16 chains for accuracy; PE
    has mid-stream slack). The final tiles go straight to PE.
  - Epilogue: per-512-block ACT sqrt with accum_out producing the block
    sums in the same instruction, one tiny DVE reduce, DMA out.
"""

import numpy as np

# Full problem shape (hardcoded per the harness contract).
R = 8192          # rows
C_FULL = 16384    # columns
N_CORES = 8
C = C_FULL // N_CORES  # 2048 columns per core
P = 128           # SBUF partitions
NBLK = 512        # matmul moving free dim (one PSUM bank of fp32)

T4 = 15           # bulk tiles: [P, 4, C], rows 0..7680
ROWS4 = T4 * P * 4
# DVE-accumulator fold points (after the adds at tile t) and the tiles
# that restart the accumulator with a copy.
FOLD_TILES = (4, 9, T4 - 1)
RESET_TILES = (5, 10)

_cached = None


def _build():
    """Build + schedule the per-core Bass program. Returns the Bacc object."""
    import concourse.bacc as bacc
    import concourse.tile as tile
    from concourse import mybir

    nc = bacc.Bacc(
        "TRN2",
        target_bir_lowering=False,
        debug=False,
        enable_asserts=False,
        num_devices=N_CORES,
    )

    s_dram = nc.dram_tensor("S", [R, C], mybir.dt.float32, kind="ExternalInput")
    out_dram = nc.dram_tensor("out", [1, 1], mybir.dt.float32, kind="ExternalOutput")

    s_ap = s_dram.ap()
    out_ap = out_dram.ap()

    # Bulk view [T4, P, 4, C]: partition p holds 4 consecutive rows ->
    # 32 KiB contiguous DRAM per (t, p) descriptor.
    v4 = s_ap[:ROWS4, :].rearrange("(t p q) c -> t p q c", p=P, q=4)
    # Step-down tail: one [P, 2, C] tile then two [P, C] sub-tiles.
    v2 = s_ap[ROWS4 : ROWS4 + 2 * P, :].rearrange("(p q) c -> p q c", q=2)
    v1 = s_ap[ROWS4 + 2 * P :, :].rearrange("(s p) c -> s p c", p=P)

    with tile.TileContext(nc) as tc:
        with (
            tc.tile_pool(name="io", bufs=4) as io_pool,
            tc.tile_pool(name="sqp", bufs=3) as sq_pool,
            tc.tile_pool(name="const", bufs=1) as const_pool,
            tc.tile_pool(name="ps", bufs=1, space="PSUM") as ps_pool,
            tc.tile_pool(name="fin", bufs=1) as fin_pool,
        ):
            # First input DMA before any const setup so streaming starts as
            # early as possible.
            x0 = io_pool.tile([P, 4, C], mybir.dt.float32, tag="x")
            nc.sync.dma_start(out=x0, in_=v4[0])

            ones = const_pool.tile([P, 1], mybir.dt.bfloat16)
            nc.vector.memset(ones, 1.0)

            # DVE-side accumulator for q=2,3 row-slices.
            acc = const_pool.tile([P, C], mybir.dt.bfloat16)

            # Per-column sum of squares (4 PSUM banks).
            colsq = ps_pool.tile([1, C], mybir.dt.float32)

            # Dummy sqrt: pulls the sqrt ACT-table load out of the tail.
            warm = const_pool.tile([1, 1], mybir.dt.float32)
            nc.scalar.sqrt(out=warm, in_=ones[0:1, :])

            def pe_reduce(src, first, last):
                for b in range(C // NBLK):
                    nc.tensor.matmul(
                        colsq[:, b * NBLK : (b + 1) * NBLK],
                        ones,
                        src[:, b * NBLK : (b + 1) * NBLK],
                        start=first,
                        stop=(last and b == C // NBLK - 1),
                    )

            for t in range(T4):
                if t == 0:
                    x_tile = x0
                else:
                    x_tile = io_pool.tile([P, 4, C], mybir.dt.float32, tag="x")
                    nc.sync.dma_start(out=x_tile, in_=v4[t])

                sq = sq_pool.tile([P, 4, C], mybir.dt.bfloat16, tag="sq")
                nc.scalar.square(out=sq, in_=x_tile)

                pe_reduce(sq[:, 0, :], first=(t == 0), last=False)
                pe_reduce(sq[:, 1, :], first=False, last=False)

                if t == 0 or t in RESET_TILES:
                    nc.vector.tensor_copy(acc, sq[:, 2, :])
                else:
                    nc.vector.tensor_add(acc, acc, sq[:, 2, :])
                nc.vector.tensor_add(acc, acc, sq[:, 3, :])

                if t in FOLD_TILES:
                    pe_reduce(acc, first=False, last=False)

            # Step-down tail (all-PE; the accumulator is already folded).
            x2 = io_pool.tile([P, 2, C], mybir.dt.float32, tag="x")
            nc.sync.dma_start(out=x2, in_=v2)
            sq2 = sq_pool.tile([P, 2, C], mybir.dt.bfloat16, tag="sq")
            nc.scalar.square(out=sq2, in_=x2)
            pe_reduce(sq2[:, 0, :], first=False, last=False)
            pe_reduce(sq2[:, 1, :], first=False, last=False)

            for s in range(2):
                xs = io_pool.tile([P, 1, C], mybir.dt.float32, tag="x")
                nc.sync.dma_start(out=xs[:, 0, :], in_=v1[s])
                sqs = sq_pool.tile([P, 1, C], mybir.dt.bfloat16, tag="sq")
                nc.scalar.square(out=sqs, in_=xs)
                pe_reduce(sqs[:, 0, :], first=False, last=(s == 1))

            # Per-block sqrt; accum_out yields each block's sum of norms in
            # the same ACT instruction.
            norms = fin_pool.tile([1, C], mybir.dt.float32)
            part = fin_pool.tile([1, C // NBLK], mybir.dt.float32)
            for b in range(C // NBLK):
                blk = slice(b * NBLK, (b + 1) * NBLK)
                nc.scalar.activation(
                    norms[:, blk],
                    colsq[:, blk],
                    mybir.ActivationFunctionType.Sqrt,
                    accum_out=part[:, b : b + 1],
                )

            total = fin_pool.tile([1, 1], mybir.dt.float32)
            nc.vector.reduce_sum(out=total, in_=part, axis=mybir.AxisListType.X)

            nc.sync.dma_start(out=out_ap, in_=total)

    nc.compile()
    return nc


def _get_nc():
    global _cached
    if _cached is None:
        _cached = _build()
    return _cached


def _run(S: np.ndarray, trace: bool = False):
    from concourse import bass_utils

    assert S.shape == (R, C_FULL), S.shape
    S = np.ascontiguousarray(np.asarray(S, dtype=np.float32))

    nc = _get_nc()
    in_maps = [
        {"S": np.ascontiguousarray(S[:, i * C : (i + 1) * C])} for i in range(N_CORES)
    ]
    try:
        res = bass_utils.run_bass_kernel_spmd(
            nc, in_maps, core_ids=list(range(N_CORES)), trace=trace
        )
    except Exception:
        # One retry: transient NRT/device hiccups (e.g. a wedged core from a
        # previous process) are recoverable on re-execution.
        res = bass_utils.run_bass_kernel_spmd(
            nc, in_maps, core_ids=list(range(N_CORES)), trace=trace
        )
    partials = np.array(
        [res.results[i]["out"][0, 0] for i in range(N_CORES)], dtype=np.float64
    )
    out = np.float32(partials.sum())
    return out, res


def kernel(S: np.ndarray) -> np.ndarray:
    out, _ = _run(S, trace=False)
    return np.asarray(out, dtype=np.float32)


def run_traced(S: np.ndarray):
    """For test.py: returns (output, BassKernelResults) with NTFF trace."""
    return _run(S, trace=True)
